# revision 22
# baseline (speedup 1.0000x reference)
"""Linear attention (non-causal, elu+1 feature map) on 8 Trainium2 cores — v8.

Math per (batch b, head h), phi(x) = elu(x)+1:
    C_aug = phi(K)^T @ [V | 1]        # (64, 65): context + k_sum col
    numer = phi(Q) @ C_aug[:, :64]
    denom = phi(Q) @ C_aug[:, 64]
    out   = numer / denom             # eps=1e-6 negligible vs denom ~1e5

Key choices vs the fp32 baseline (233us):
  * fp16 inputs (host casts): PE matmuls at 1 cycle/row instead of 4, one
    LDWEIGHTS pass instead of two, half the HBM traffic (33MB -> 16.3MB per
    core).
  * Both heads fused per matmul. Host packs [K0|K1|V0|1|V1|1] (258 cols per
    t-row) so mm1's stationary (128 K-cols) and moving (130 V-cols) APs are
    single-stride; psum diag blocks give C0_aug/C1_aug. mm2 streams a
    block-diagonal 128x130 C against contiguous 128-col phiQ chunks.
  * phi split balanced across PE and DVE (both measured near-saturated):
      - K: phi materialized in place via min (DVE 4x) / exp (Act) /
        (x+1)max(t) scalar_tensor_tensor (DVE 1x) -> mm1 is 32 matmuls.
      - Q: phi(q) = exp(min(q,0)) + relu(q), never materialized; mm2
        accumulates the E and R passes in psum (2 matmuls/chunk), keeping
        the 1x-rate stt off the DVE for Q at the cost of PE time.
  * Per-queue DMA bandwidth is only ~24GB/s, so every transfer is split into
    ~130-260KB pieces across many queues (input triggers on SP, output
    triggers on the idle Pool DGE so they can't head-of-line block input
    prefetch). Batch 0 is split finer to shorten pipeline fill.
  * normalize: reciprocal_approx_fast (51 ULP, ~5x faster; denom ~1e5 so
    edge cases are impossible) + one stride-0-broadcast scalar_tensor_tensor
    per 3-chunk psum group, streaming each finished group to HBM.
  * Three-stage software pipeline in EMISSION order (engine sequencers are
    in-order): L(b)=loads+phi, M(b)=mm1, B(b)=C-cast+mm2+normalize+output,
    emitted L0 M0 L1 B0 M1 L2 B1 M2 L3 B2 M3 B3: PE order stays
    mm1(b),mm2(b),mm1(b+1); DVE does phi(b+1) before norm(b); Act does
    exp(b+1) before the C-cast of b.

Accuracy: fp16 quantization of phi(K),V gives C entries ~0.2% rms error;
through the normalizer this lands ~1.4e-4 absolute worst-case on outputs vs
the 2e-2 per-element gate with its 1e-3 floor (measured 1.52e-2 max rel).
"""

from contextlib import ExitStack

import numpy as np

import concourse.bacc as bacc
import concourse.bass as bass
import concourse.mybir as mybir
import concourse.tile as tile
from concourse.bass_utils import run_bass_kernel_spmd

B = 4
T = 4096
D = 1024
H = 16
E = 64
EA = E + 1
NCORES = 8
HPC = H // NCORES  # 2 heads per core
KC = HPC * E  # 128 packed K columns per t-row
W2 = KC + HPC * EA  # 258 cols per kva row: [K0|K1|V0|1|V1|1]
P = 128
NT = T // P  # 32 t-tiles for mm1 (t = p*32 + n)
NJ = T // P  # 32 t-chunks for mm2 (t = 128*j + p)
F16 = mybir.dt.float16
F32 = mybir.dt.float32
BF16 = mybir.dt.bfloat16
AF = mybir.ActivationFunctionType
ALU = mybir.AluOpType

# mm2 psum grouping: chunks per tile (3*130*4B = 1560B <= 2KB bank)
GRPS = [3, 3, 2, 3, 3, 2, 3, 3, 2, 3, 3, 2]
assert sum(GRPS) == NJ


def build_nc():
    nc = bacc.Bacc("TRN2", target_bir_lowering=False, debug=False)
    qt = nc.dram_tensor("qt", [B, P, T], F16, kind="ExternalInput").ap()
    kva = nc.dram_tensor("kva", [B, T, W2], F16, kind="ExternalInput").ap()
    o = nc.dram_tensor("o", [B, P, NJ * HPC * E], BF16, kind="ExternalOutput").ap()

    with tile.TileContext(nc) as tc, ExitStack() as ctx:
        qt_pool = ctx.enter_context(tc.tile_pool(name="qt", bufs=3))
        kv_pool = ctx.enter_context(tc.tile_pool(name="kv", bufs=2))
        eq_pool = ctx.enter_context(tc.tile_pool(name="eq", bufs=12))
        tk_pool = ctx.enter_context(tc.tile_pool(name="tk", bufs=8))
        c_pool = ctx.enter_context(tc.tile_pool(name="c", bufs=2))
        out_pool = ctx.enter_context(tc.tile_pool(name="out", bufs=2))
        r_pool = ctx.enter_context(tc.tile_pool(name="r", bufs=8))
        psc_pool = ctx.enter_context(tc.tile_pool(name="psc", bufs=2, space="PSUM"))
        pso_pool = ctx.enter_context(tc.tile_pool(name="pso", bufs=6, space="PSUM"))

        HW = NT * W2  # 8256 elems per partition
        TQ = T // 4  # 1024 cols per phi quarter
        NQ = NT // 4  # 8 n-tiles per phi quarter

        state = {}

        def emit_load_phi(b):
            # Q^T load; E_q = exp(min(q,0)) into eq tiles, R_q = relu(q).
            # DMA pieces sized ~130KB (b=0: finer) so single-queue time stays
            # low; each phi quarter depends only on its own pieces.
            # K side first: mm1 is the earliest PE consumer, so its DMAs and
            # phi must lead the in-order SP/DVE/Act queues.
            # [K0|K1|V0|1|V1|1] load. Even quarters: phi(K) materialized in
            # place via stt (mm1 single pass). Odd quarters: E_k in tk, R_k
            # in place (mm1 double pass) — balances DVE vs PE load.
            kv = kv_pool.tile([P, HW], F16)
            kvr = kv[:].rearrange("p (n c) -> p n c", c=W2)
            tks = {}
            for q4 in range(4):
                # 2 pieces/quarter: single-queue ~5.5us (4 on the very first
                # quarter so the first mm1 weights land sooner)
                npz = 4 if (b == 0 and q4 == 0) else 2
                for z in range(npz):
                    w = HW // 4 // npz
                    csl = slice(q4 * (HW // 4) + z * w, q4 * (HW // 4) + (z + 1) * w)
                    nc.sync.dma_start(
                        kv[:, csl],
                        kva[b].rearrange("(p n) c -> p (n c)", p=P)[:, csl],
                    )
                nsl = slice(q4 * NQ, (q4 + 1) * NQ)
                kview = kvr[:, nsl, 0:KC]
                tk = tk_pool.tile([P, NQ * KC], F16)
                tk3 = tk[:].rearrange("p (n c) -> p n c", c=KC)
                nc.vector.tensor_scalar_min(tk3, kview, 0.0)
                nc.scalar.activation(tk3, tk3, AF.Exp)
                if q4 % 2 == 0:
                    nc.vector.scalar_tensor_tensor(
                        kview, kview, 1.0, tk3, ALU.add, ALU.max
                    )
                else:
                    nc.vector.tensor_scalar_max(kview, kview, 0.0)
                    tks[q4] = tk

            qt_t = qt_pool.tile([P, T], F16)
            eqs = []
            for q4 in range(4):
                for z in range(2):
                    w = TQ // 2
                    sl = slice(q4 * TQ + z * w, q4 * TQ + (z + 1) * w)
                    nc.sync.dma_start(qt_t[:, sl], qt[b, :, sl])
                sl = slice(q4 * TQ, (q4 + 1) * TQ)
                x = qt_t[:, sl]
                tq = eq_pool.tile([P, TQ], F16)
                nc.vector.tensor_scalar_min(tq[:], x, 0.0)
                nc.scalar.activation(tq[:], tq[:], AF.Exp)
                nc.vector.tensor_scalar_max(x, x, 0.0)
                eqs.append(tq)
            state[b] = (qt_t, eqs, kv, kvr, tks)

        def emit_mm1(b):
            qt_t, eqs, kv, kvr, tks = state[b]
            psc = psc_pool.tile([P, HPC * EA], F32)
            for n in range(NT):
                q4, nq = n // NQ, n % NQ
                if q4 % 2 == 1:
                    nc.tensor.matmul(
                        psc[:],
                        lhsT=tks[q4][:, nq * KC : (nq + 1) * KC],
                        rhs=kvr[:, n, KC:W2],
                        start=(n == 0),
                        stop=False,
                    )
                nc.tensor.matmul(
                    psc[:],
                    lhsT=kvr[:, n, 0:KC],
                    rhs=kvr[:, n, KC:W2],
                    start=(n == 0),  # n=0 is in an even (single-pass) quarter
                    stop=(n == NT - 1),
                )
            # Cast the C diag blocks right here: Act reaches these just as
            # mm1 drains, instead of after the NEXT batch's exp passes, which
            # kept mm2 waiting ~5us per batch on c_sb.
            c_sb = c_pool.tile([P, HPC * EA], F16)
            nc.vector.memset(c_sb[:], 0.0)
            nc.scalar.copy(c_sb[0:E, 0:EA], psc[0:E, 0:EA])
            nc.vector.tensor_copy(c_sb[E:P, EA : 2 * EA], psc[E:P, EA : 2 * EA])
            state[b] = (qt_t, eqs, c_sb)

        def emit_tail(b):
            qt_t, eqs, c_sb = state[b]
            # mm2 (E and R accumulated) + normalize + streamed output
            ob = out_pool.tile([P, NJ * HPC * E], BF16)
            j = 0
            for gi, grp in enumerate(GRPS):
                ps = pso_pool.tile([P, grp * HPC * EA], F32)
                for k in range(grp):
                    jj = j + k
                    q4, jq = jj // 8, jj % 8
                    blk = ps[:, k * HPC * EA : (k + 1) * HPC * EA]
                    nc.tensor.matmul(
                        blk,
                        lhsT=eqs[q4][:, jq * P : (jq + 1) * P],
                        rhs=c_sb[:],
                        start=True,
                        stop=False,
                    )
                    nc.tensor.matmul(
                        blk,
                        lhsT=qt_t[:, jj * P : (jj + 1) * P],
                        rhs=c_sb[:],
                        start=False,
                        stop=True,
                    )
                r = r_pool.tile([P, grp * HPC], F32)
                nc.vector.reciprocal_approx_fast(r[:], ps[:, E::EA])
                numer = ps[:].rearrange("p (k h c) -> p k h c", k=grp, h=HPC)[
                    :, :, :, 0:E
                ]
                rb = r[:].rearrange("p (k h c) -> p k h c", k=grp, h=HPC)
                numer_b, rb = bass.broadcast_tensor_aps(numer, rb)
                osl = slice(j * HPC * E, (j + grp) * HPC * E)
                oview = ob[:, osl].rearrange("p (k h c) -> p k h c", k=grp, h=HPC)
                nc.vector.scalar_tensor_tensor(
                    oview, numer_b, 1.0, rb, ALU.mult, ALU.mult
                )
                # stream output on the Pool DGE queue: each trigger costs
                # ~600ns of Pool-engine SWDGE time, so keep pieces coarse
                # (one per 8-chunk span) except on the last batch, where
                # smaller per-group pieces shorten the drain.
                if b == B - 1:
                    # halve the final pieces: the very last transfer's
                    # single-queue time is pure drain
                    w = (osl.stop - osl.start) // 2
                    nc.gpsimd.dma_start(
                        o[b][:, osl.start : osl.start + w],
                        ob[:, osl.start : osl.start + w],
                    )
                    nc.gpsimd.dma_start(
                        o[b][:, osl.start + w : osl.stop],
                        ob[:, osl.start + w : osl.stop],
                    )
                elif gi % 3 == 2:
                    qsl = slice((j + grp - 8) * HPC * E, (j + grp) * HPC * E)
                    nc.gpsimd.dma_start(o[b][:, qsl], ob[:, qsl])
                j += grp

        emit_load_phi(0)
        emit_mm1(0)
        emit_load_phi(1)
        emit_tail(0)
        emit_mm1(1)
        emit_load_phi(2)
        emit_tail(1)
        emit_mm1(2)
        emit_load_phi(3)
        emit_tail(2)
        emit_mm1(3)
        emit_tail(3)
    nc.finalize()
    return nc


_NC_CACHE = None


def _get_nc():
    global _NC_CACHE
    if _NC_CACHE is None:
        _NC_CACHE = build_nc()
    return _NC_CACHE


def make_in_maps(query, key, value):
    query = np.asarray(query, dtype=np.float32)
    key = np.asarray(key, dtype=np.float32)
    value = np.asarray(value, dtype=np.float32)
    in_maps = []
    for c in range(NCORES):
        lo = c * HPC * E
        hi = lo + HPC * E
        qt = np.ascontiguousarray(
            query[:, :, lo:hi].transpose(0, 2, 1), dtype=np.float16
        )
        kva = np.empty((B, T, W2), np.float16)
        kva[..., 0:KC] = key[:, :, lo:hi]
        kva[..., KC : KC + E] = value[:, :, lo : lo + E]
        kva[..., KC + E] = 1.0
        kva[..., KC + EA : KC + EA + E] = value[:, :, lo + E : hi]
        kva[..., KC + EA + E] = 1.0
        in_maps.append({"qt": qt, "kva": kva})
    return in_maps


def assemble_out(results):
    out = np.empty((B, T, D), np.float32)
    for c in range(NCORES):
        # o[b, p, ((j*2 + h)*64 + e)] = out[b, t=128j+p, c*128 + h*64 + e]
        oc = np.asarray(results[c]["o"], dtype=np.float32)
        oc = oc.reshape(B, P, NJ, HPC, E).transpose(0, 2, 1, 3, 4)
        out[:, :, c * HPC * E : (c + 1) * HPC * E] = oc.reshape(B, T, HPC * E)
    return out


def run(query, key, value, **spmd_kwargs):
    nc = _get_nc()
    in_maps = make_in_maps(query, key, value)
    res = run_bass_kernel_spmd(nc, in_maps, core_ids=list(range(NCORES)), **spmd_kwargs)
    return assemble_out(res.results), res


def kernel(query, key, value):
    out, _ = run(query, key, value)
    return out


# revision 23
# speedup vs baseline: 1.0537x; 1.0537x over previous
"""Linear attention (non-causal, elu+1 feature map) on 8 Trainium2 cores — v8.

Math per (batch b, head h), phi(x) = elu(x)+1:
    C_aug = phi(K)^T @ [V | 1]        # (64, 65): context + k_sum col
    numer = phi(Q) @ C_aug[:, :64]
    denom = phi(Q) @ C_aug[:, 64]
    out   = numer / denom             # eps=1e-6 negligible vs denom ~1e5

Key choices vs the fp32 baseline (233us):
  * fp16 inputs (host casts): PE matmuls at 1 cycle/row instead of 4, one
    LDWEIGHTS pass instead of two, half the HBM traffic (33MB -> 16.3MB per
    core).
  * Both heads fused per matmul. Host packs [K0|K1|V0|1|V1|1] (258 cols per
    t-row) so mm1's stationary (128 K-cols) and moving (130 V-cols) APs are
    single-stride; psum diag blocks give C0_aug/C1_aug. mm2 streams a
    block-diagonal 128x130 C against contiguous 128-col phiQ chunks.
  * phi split balanced across PE and DVE (both measured near-saturated):
      - K: phi materialized in place via min (DVE 4x) / exp (Act) /
        (x+1)max(t) scalar_tensor_tensor (DVE 1x) -> mm1 is 32 matmuls.
      - Q: phi(q) = exp(min(q,0)) + relu(q), never materialized; mm2
        accumulates the E and R passes in psum (2 matmuls/chunk), keeping
        the 1x-rate stt off the DVE for Q at the cost of PE time.
  * Per-queue DMA bandwidth is only ~24GB/s, so every transfer is split into
    ~130-260KB pieces across many queues (input triggers on SP, output
    triggers on the idle Pool DGE so they can't head-of-line block input
    prefetch). Batch 0 is split finer to shorten pipeline fill.
  * normalize: reciprocal_approx_fast (51 ULP, ~5x faster; denom ~1e5 so
    edge cases are impossible) + one stride-0-broadcast scalar_tensor_tensor
    per 3-chunk psum group, streaming each finished group to HBM.
  * Three-stage software pipeline in EMISSION order (engine sequencers are
    in-order): L(b)=loads+phi, M(b)=mm1, B(b)=C-cast+mm2+normalize+output,
    emitted L0 M0 L1 B0 M1 L2 B1 M2 L3 B2 M3 B3: PE order stays
    mm1(b),mm2(b),mm1(b+1); DVE does phi(b+1) before norm(b); Act does
    exp(b+1) before the C-cast of b.

Accuracy: fp16 quantization of phi(K),V gives C entries ~0.2% rms error;
through the normalizer this lands ~1.4e-4 absolute worst-case on outputs vs
the 2e-2 per-element gate with its 1e-3 floor (measured 1.52e-2 max rel).
"""

from contextlib import ExitStack

import numpy as np

import concourse.bacc as bacc
import concourse.bass as bass
import concourse.mybir as mybir
import concourse.tile as tile
from concourse.bass_utils import run_bass_kernel_spmd

B = 4
T = 4096
D = 1024
H = 16
E = 64
EA = E + 1
NCORES = 8
HPC = H // NCORES  # 2 heads per core
KC = HPC * E  # 128 packed K columns per t-row
W2 = KC + HPC * EA  # 258 cols per kva row: [K0|K1|V0|1|V1|1]
P = 128
NT = T // P  # 32 t-tiles for mm1 (t = p*32 + n)
NJ = T // P  # 32 t-chunks for mm2 (t = 128*j + p)
F16 = mybir.dt.float16
F32 = mybir.dt.float32
BF16 = mybir.dt.bfloat16
AF = mybir.ActivationFunctionType
ALU = mybir.AluOpType

# mm2 psum grouping: chunks per tile (3*130*4B = 1560B <= 2KB bank)
GRPS = [3, 3, 2, 3, 3, 2, 3, 3, 2, 3, 3, 2]
assert sum(GRPS) == NJ


def build_nc():
    nc = bacc.Bacc("TRN2", target_bir_lowering=False, debug=False)
    qt = nc.dram_tensor("qt", [B, P, T], F16, kind="ExternalInput").ap()
    kva = nc.dram_tensor("kva", [B, T, W2], F16, kind="ExternalInput").ap()
    o = nc.dram_tensor("o", [B, P, NJ * HPC * E], BF16, kind="ExternalOutput").ap()

    with tile.TileContext(nc) as tc, ExitStack() as ctx:
        qt_pool = ctx.enter_context(tc.tile_pool(name="qt", bufs=3))
        kv_pool = ctx.enter_context(tc.tile_pool(name="kv", bufs=2))
        eq_pool = ctx.enter_context(tc.tile_pool(name="eq", bufs=12))
        tk_pool = ctx.enter_context(tc.tile_pool(name="tk", bufs=8))
        c_pool = ctx.enter_context(tc.tile_pool(name="c", bufs=2))
        out_pool = ctx.enter_context(tc.tile_pool(name="out", bufs=2))
        r_pool = ctx.enter_context(tc.tile_pool(name="r", bufs=8))
        psc_pool = ctx.enter_context(tc.tile_pool(name="psc", bufs=2, space="PSUM"))
        pso_pool = ctx.enter_context(tc.tile_pool(name="pso", bufs=6, space="PSUM"))

        HW = NT * W2  # 8256 elems per partition
        TQ = T // 4  # 1024 cols per phi quarter
        NQ = NT // 4  # 8 n-tiles per phi quarter

        state = {}

        def emit_load_phi(b):
            # Q^T load; E_q = exp(min(q,0)) into eq tiles, R_q = relu(q).
            # DMA pieces sized ~130KB (b=0: finer) so single-queue time stays
            # low; each phi quarter depends only on its own pieces.
            # K side first: mm1 is the earliest PE consumer, so its DMAs and
            # phi must lead the in-order SP/DVE/Act queues.
            # [K0|K1|V0|1|V1|1] load. Even quarters: phi(K) materialized in
            # place via stt (mm1 single pass). Odd quarters: E_k in tk, R_k
            # in place (mm1 double pass) — balances DVE vs PE load.
            kv = kv_pool.tile([P, HW], F16)
            kvr = kv[:].rearrange("p (n c) -> p n c", c=W2)
            tks = {}
            for q4 in range(4):
                # 2 pieces/quarter: single-queue ~5.5us (4 on the very first
                # quarter so the first mm1 weights land sooner)
                npz = 4 if (b == 0 and q4 == 0) else 2
                for z in range(npz):
                    w = HW // 4 // npz
                    csl = slice(q4 * (HW // 4) + z * w, q4 * (HW // 4) + (z + 1) * w)
                    nc.sync.dma_start(
                        kv[:, csl],
                        kva[b].rearrange("(p n) c -> p (n c)", p=P)[:, csl],
                    )
                nsl = slice(q4 * NQ, (q4 + 1) * NQ)
                kview = kvr[:, nsl, 0:KC]
                tk = tk_pool.tile([P, NQ * KC], F16)
                tk3 = tk[:].rearrange("p (n c) -> p n c", c=KC)
                nc.vector.tensor_scalar_min(tk3, kview, 0.0)
                nc.scalar.activation(tk3, tk3, AF.Exp)
                if q4 % 2 == 0:
                    nc.vector.scalar_tensor_tensor(
                        kview, kview, 1.0, tk3, ALU.add, ALU.max
                    )
                else:
                    nc.vector.tensor_scalar_max(kview, kview, 0.0)
                    tks[q4] = tk

            qt_t = qt_pool.tile([P, T], F16)
            eqs = []
            for q4 in range(4):
                for z in range(2):
                    w = TQ // 2
                    sl = slice(q4 * TQ + z * w, q4 * TQ + (z + 1) * w)
                    nc.sync.dma_start(qt_t[:, sl], qt[b, :, sl])
                sl = slice(q4 * TQ, (q4 + 1) * TQ)
                x = qt_t[:, sl]
                tq = eq_pool.tile([P, TQ], F16)
                nc.vector.tensor_scalar_min(tq[:], x, 0.0)
                nc.scalar.activation(tq[:], tq[:], AF.Exp)
                nc.vector.tensor_scalar_max(x, x, 0.0)
                eqs.append(tq)
            state[b] = (qt_t, eqs, kv, kvr, tks)

        def emit_mm1(b):
            qt_t, eqs, kv, kvr, tks = state[b]
            psc = psc_pool.tile([P, HPC * EA], F32)
            for n in range(NT):
                q4, nq = n // NQ, n % NQ
                if q4 % 2 == 1:
                    nc.tensor.matmul(
                        psc[:],
                        lhsT=tks[q4][:, nq * KC : (nq + 1) * KC],
                        rhs=kvr[:, n, KC:W2],
                        start=(n == 0),
                        stop=False,
                    )
                nc.tensor.matmul(
                    psc[:],
                    lhsT=kvr[:, n, 0:KC],
                    rhs=kvr[:, n, KC:W2],
                    start=(n == 0),  # n=0 is in an even (single-pass) quarter
                    stop=(n == NT - 1),
                )
            # Cast the C diag blocks right here: Act reaches these just as
            # mm1 drains, instead of after the NEXT batch's exp passes, which
            # kept mm2 waiting ~5us per batch on c_sb.
            c_sb = c_pool.tile([P, HPC * EA], F16)
            nc.vector.memset(c_sb[:], 0.0)
            nc.scalar.copy(c_sb[0:E, 0:EA], psc[0:E, 0:EA])
            nc.scalar.copy(c_sb[E:P, EA : 2 * EA], psc[E:P, EA : 2 * EA])
            state[b] = (qt_t, eqs, c_sb)

        def emit_tail(b):
            qt_t, eqs, c_sb = state[b]
            # mm2 (E and R accumulated) + normalize + streamed output
            ob = out_pool.tile([P, NJ * HPC * E], BF16)
            j = 0
            for gi, grp in enumerate(GRPS):
                ps = pso_pool.tile([P, grp * HPC * EA], F32)
                for k in range(grp):
                    jj = j + k
                    q4, jq = jj // 8, jj % 8
                    blk = ps[:, k * HPC * EA : (k + 1) * HPC * EA]
                    nc.tensor.matmul(
                        blk,
                        lhsT=eqs[q4][:, jq * P : (jq + 1) * P],
                        rhs=c_sb[:],
                        start=True,
                        stop=False,
                    )
                    nc.tensor.matmul(
                        blk,
                        lhsT=qt_t[:, jj * P : (jj + 1) * P],
                        rhs=c_sb[:],
                        start=False,
                        stop=True,
                    )
                r = r_pool.tile([P, grp * HPC], F32)
                nc.vector.reciprocal_approx_fast(r[:], ps[:, E::EA])
                numer = ps[:].rearrange("p (k h c) -> p k h c", k=grp, h=HPC)[
                    :, :, :, 0:E
                ]
                rb = r[:].rearrange("p (k h c) -> p k h c", k=grp, h=HPC)
                numer_b, rb = bass.broadcast_tensor_aps(numer, rb)
                osl = slice(j * HPC * E, (j + grp) * HPC * E)
                oview = ob[:, osl].rearrange("p (k h c) -> p k h c", k=grp, h=HPC)
                nc.vector.scalar_tensor_tensor(
                    oview, numer_b, 1.0, rb, ALU.mult, ALU.mult
                )
                # stream output on the Pool DGE queue: each trigger costs
                # ~600ns of Pool-engine SWDGE time, so keep pieces coarse
                # (one per 8-chunk span) except on the last batch, where
                # smaller per-group pieces shorten the drain.
                if b == B - 1:
                    # halve the final pieces: the very last transfer's
                    # single-queue time is pure drain
                    w = (osl.stop - osl.start) // 2
                    nc.gpsimd.dma_start(
                        o[b][:, osl.start : osl.start + w],
                        ob[:, osl.start : osl.start + w],
                    )
                    nc.gpsimd.dma_start(
                        o[b][:, osl.start + w : osl.stop],
                        ob[:, osl.start + w : osl.stop],
                    )
                elif gi % 3 == 2:
                    qsl = slice((j + grp - 8) * HPC * E, (j + grp) * HPC * E)
                    nc.gpsimd.dma_start(o[b][:, qsl], ob[:, qsl])
                j += grp

        emit_load_phi(0)
        emit_mm1(0)
        emit_load_phi(1)
        emit_tail(0)
        emit_mm1(1)
        emit_load_phi(2)
        emit_tail(1)
        emit_mm1(2)
        emit_load_phi(3)
        emit_tail(2)
        emit_mm1(3)
        emit_tail(3)
    nc.finalize()
    return nc


_NC_CACHE = None


def _get_nc():
    global _NC_CACHE
    if _NC_CACHE is None:
        _NC_CACHE = build_nc()
    return _NC_CACHE


def make_in_maps(query, key, value):
    query = np.asarray(query, dtype=np.float32)
    key = np.asarray(key, dtype=np.float32)
    value = np.asarray(value, dtype=np.float32)
    in_maps = []
    for c in range(NCORES):
        lo = c * HPC * E
        hi = lo + HPC * E
        qt = np.ascontiguousarray(
            query[:, :, lo:hi].transpose(0, 2, 1), dtype=np.float16
        )
        kva = np.empty((B, T, W2), np.float16)
        kva[..., 0:KC] = key[:, :, lo:hi]
        kva[..., KC : KC + E] = value[:, :, lo : lo + E]
        kva[..., KC + E] = 1.0
        kva[..., KC + EA : KC + EA + E] = value[:, :, lo + E : hi]
        kva[..., KC + EA + E] = 1.0
        in_maps.append({"qt": qt, "kva": kva})
    return in_maps


def assemble_out(results):
    out = np.empty((B, T, D), np.float32)
    for c in range(NCORES):
        # o[b, p, ((j*2 + h)*64 + e)] = out[b, t=128j+p, c*128 + h*64 + e]
        oc = np.asarray(results[c]["o"], dtype=np.float32)
        oc = oc.reshape(B, P, NJ, HPC, E).transpose(0, 2, 1, 3, 4)
        out[:, :, c * HPC * E : (c + 1) * HPC * E] = oc.reshape(B, T, HPC * E)
    return out


def run(query, key, value, **spmd_kwargs):
    nc = _get_nc()
    in_maps = make_in_maps(query, key, value)
    res = run_bass_kernel_spmd(nc, in_maps, core_ids=list(range(NCORES)), **spmd_kwargs)
    return assemble_out(res.results), res


def kernel(query, key, value):
    out, _ = run(query, key, value)
    return out


# revision 25
# speedup vs baseline: 1.1347x; 1.0769x over previous
"""Linear attention (non-causal, elu+1 feature map) on 8 Trainium2 cores — v8.

Math per (batch b, head h), phi(x) = elu(x)+1:
    C_aug = phi(K)^T @ [V | 1]        # (64, 65): context + k_sum col
    numer = phi(Q) @ C_aug[:, :64]
    denom = phi(Q) @ C_aug[:, 64]
    out   = numer / denom             # eps=1e-6 negligible vs denom ~1e5

Key choices vs the fp32 baseline (233us):
  * fp16 inputs (host casts): PE matmuls at 1 cycle/row instead of 4, one
    LDWEIGHTS pass instead of two, half the HBM traffic (33MB -> 16.3MB per
    core).
  * Both heads fused per matmul. Host packs [K0|K1|V0|1|V1|1] (258 cols per
    t-row) so mm1's stationary (128 K-cols) and moving (130 V-cols) APs are
    single-stride; psum diag blocks give C0_aug/C1_aug. mm2 streams a
    block-diagonal 128x130 C against contiguous 128-col phiQ chunks.
  * phi split balanced across PE and DVE (both measured near-saturated):
      - K: phi materialized in place via min (DVE 4x) / exp (Act) /
        (x+1)max(t) scalar_tensor_tensor (DVE 1x) -> mm1 is 32 matmuls.
      - Q: phi(q) = exp(min(q,0)) + relu(q), never materialized; mm2
        accumulates the E and R passes in psum (2 matmuls/chunk), keeping
        the 1x-rate stt off the DVE for Q at the cost of PE time.
  * Per-queue DMA bandwidth is only ~24GB/s, so every transfer is split into
    ~130-260KB pieces across many queues (input triggers on SP, output
    triggers on the idle Pool DGE so they can't head-of-line block input
    prefetch). Batch 0 is split finer to shorten pipeline fill.
  * normalize: reciprocal_approx_fast (51 ULP, ~5x faster; denom ~1e5 so
    edge cases are impossible) + one stride-0-broadcast scalar_tensor_tensor
    per 3-chunk psum group, streaming each finished group to HBM.
  * Three-stage software pipeline in EMISSION order (engine sequencers are
    in-order): L(b)=loads+phi, M(b)=mm1, B(b)=C-cast+mm2+normalize+output,
    emitted L0 M0 L1 B0 M1 L2 B1 M2 L3 B2 M3 B3: PE order stays
    mm1(b),mm2(b),mm1(b+1); DVE does phi(b+1) before norm(b); Act does
    exp(b+1) before the C-cast of b.

Accuracy: fp16 quantization of phi(K),V gives C entries ~0.2% rms error;
through the normalizer this lands ~1.4e-4 absolute worst-case on outputs vs
the 2e-2 per-element gate with its 1e-3 floor (measured 1.52e-2 max rel).
"""

from contextlib import ExitStack

import numpy as np

import concourse.bacc as bacc
import concourse.bass as bass
import concourse.mybir as mybir
import concourse.tile as tile
from concourse.bass_utils import run_bass_kernel_spmd

B = 4
T = 4096
D = 1024
H = 16
E = 64
EA = E + 1
NCORES = 8
HPC = H // NCORES  # 2 heads per core
KC = HPC * E  # 128 packed K columns per t-row
W2 = KC + HPC * EA  # 258 cols per kva row: [K0|K1|V0|1|V1|1]
P = 128
NT = T // P  # 32 t-tiles for mm1 (t = p*32 + n)
NJ = T // P  # 32 t-chunks for mm2 (t = 128*j + p)
F16 = mybir.dt.float16
F32 = mybir.dt.float32
BF16 = mybir.dt.bfloat16
AF = mybir.ActivationFunctionType
ALU = mybir.AluOpType

# mm2 psum grouping: chunks per tile (3*130*4B = 1560B <= 2KB bank)
GRPS = [3, 3, 2, 3, 3, 2, 3, 3, 2, 3, 3, 2]
assert sum(GRPS) == NJ


def build_nc():
    nc = bacc.Bacc("TRN2", target_bir_lowering=False, debug=False)
    qt = nc.dram_tensor("qt", [B, P, T], F16, kind="ExternalInput").ap()
    kva = nc.dram_tensor("kva", [B, T, W2], F16, kind="ExternalInput").ap()
    o = nc.dram_tensor("o", [B, P, NJ * HPC * E], BF16, kind="ExternalOutput").ap()

    with tile.TileContext(nc) as tc, ExitStack() as ctx:
        qt_pool = ctx.enter_context(tc.tile_pool(name="qt", bufs=3))
        kv_pool = ctx.enter_context(tc.tile_pool(name="kv", bufs=2))
        eq_pool = ctx.enter_context(tc.tile_pool(name="eq", bufs=12))
        tk_pool = ctx.enter_context(tc.tile_pool(name="tk", bufs=8))
        c_pool = ctx.enter_context(tc.tile_pool(name="c", bufs=2))
        out_pool = ctx.enter_context(tc.tile_pool(name="out", bufs=2))
        r_pool = ctx.enter_context(tc.tile_pool(name="r", bufs=8))
        psc_pool = ctx.enter_context(tc.tile_pool(name="psc", bufs=2, space="PSUM"))
        pso_pool = ctx.enter_context(tc.tile_pool(name="pso", bufs=6, space="PSUM"))

        HW = NT * W2  # 8256 elems per partition
        TQ = T // 4  # 1024 cols per phi quarter
        NQ = NT // 4  # 8 n-tiles per phi quarter

        state = {}

        def emit_load_phi(b):
            # Q^T load; E_q = exp(min(q,0)) into eq tiles, R_q = relu(q).
            # DMA pieces sized ~130KB (b=0: finer) so single-queue time stays
            # low; each phi quarter depends only on its own pieces.
            # K side first: mm1 is the earliest PE consumer, so its DMAs and
            # phi must lead the in-order SP/DVE/Act queues.
            # [K0|K1|V0|1|V1|1] load. Even quarters: phi(K) materialized in
            # place via stt (mm1 single pass). Odd quarters: E_k in tk, R_k
            # in place (mm1 double pass) — balances DVE vs PE load.
            kv = kv_pool.tile([P, HW], F16)
            kvr = kv[:].rearrange("p (n c) -> p n c", c=W2)
            tks = {}
            for q4 in range(4):
                for z in range(2):  # 2 pieces/quarter: single-queue ~5.5us
                    w = HW // 8
                    csl = slice(q4 * (HW // 4) + z * w, q4 * (HW // 4) + (z + 1) * w)
                    nc.sync.dma_start(
                        kv[:, csl],
                        kva[b].rearrange("(p n) c -> p (n c)", p=P)[:, csl],
                    )
                nsl = slice(q4 * NQ, (q4 + 1) * NQ)
                kview = kvr[:, nsl, 0:KC]
                tk = tk_pool.tile([P, NQ * KC], F16)
                tk3 = tk[:].rearrange("p (n c) -> p n c", c=KC)
                nc.vector.tensor_scalar_min(tk3, kview, 0.0)
                nc.scalar.activation(tk3, tk3, AF.Exp)
                if q4 % 2 == 0:
                    nc.vector.scalar_tensor_tensor(
                        kview, kview, 1.0, tk3, ALU.add, ALU.max
                    )
                else:
                    nc.vector.tensor_scalar_max(kview, kview, 0.0)
                    tks[q4] = tk

            qt_t = qt_pool.tile([P, T], F16)
            eqs = []
            for q4 in range(4):
                for z in range(2):
                    w = TQ // 2
                    sl = slice(q4 * TQ + z * w, q4 * TQ + (z + 1) * w)
                    nc.sync.dma_start(qt_t[:, sl], qt[b, :, sl])
                sl = slice(q4 * TQ, (q4 + 1) * TQ)
                x = qt_t[:, sl]
                tq = eq_pool.tile([P, TQ], F16)
                nc.vector.tensor_scalar_min(tq[:], x, 0.0)
                nc.scalar.activation(tq[:], tq[:], AF.Exp)
                nc.vector.tensor_scalar_max(x, x, 0.0)
                eqs.append(tq)
            state[b] = (qt_t, eqs, kv, kvr, tks)

        def emit_mm1(b):
            qt_t, eqs, kv, kvr, tks = state[b]
            psc = psc_pool.tile([P, HPC * EA], F32)
            for n in range(NT):
                q4, nq = n // NQ, n % NQ
                if q4 % 2 == 1:
                    nc.tensor.matmul(
                        psc[:],
                        lhsT=tks[q4][:, nq * KC : (nq + 1) * KC],
                        rhs=kvr[:, n, KC:W2],
                        start=(n == 0),
                        stop=False,
                    )
                nc.tensor.matmul(
                    psc[:],
                    lhsT=kvr[:, n, 0:KC],
                    rhs=kvr[:, n, KC:W2],
                    start=(n == 0),  # n=0 is in an even (single-pass) quarter
                    stop=(n == NT - 1),
                )
            # Cast the C diag blocks right here: Act reaches these just as
            # mm1 drains, instead of after the NEXT batch's exp passes, which
            # kept mm2 waiting ~5us per batch on c_sb.
            c_sb = c_pool.tile([P, HPC * EA], F16)
            nc.vector.memset(c_sb[:], 0.0)
            nc.scalar.copy(c_sb[0:E, 0:EA], psc[0:E, 0:EA])
            nc.scalar.copy(c_sb[E:P, EA : 2 * EA], psc[E:P, EA : 2 * EA])
            state[b] = (qt_t, eqs, c_sb)

        def emit_tail(b):
            qt_t, eqs, c_sb = state[b]
            # mm2 (E and R accumulated) + normalize + streamed output
            ob = out_pool.tile([P, NJ * HPC * E], BF16)
            j = 0
            for gi, grp in enumerate(GRPS):
                ps = pso_pool.tile([P, grp * HPC * EA], F32)
                for k in range(grp):
                    jj = j + k
                    q4, jq = jj // 8, jj % 8
                    blk = ps[:, k * HPC * EA : (k + 1) * HPC * EA]
                    nc.tensor.matmul(
                        blk,
                        lhsT=eqs[q4][:, jq * P : (jq + 1) * P],
                        rhs=c_sb[:],
                        start=True,
                        stop=False,
                    )
                    nc.tensor.matmul(
                        blk,
                        lhsT=qt_t[:, jj * P : (jj + 1) * P],
                        rhs=c_sb[:],
                        start=False,
                        stop=True,
                    )
                r = r_pool.tile([P, grp * HPC], F32)
                nc.vector.reciprocal_approx_fast(r[:], ps[:, E::EA])
                numer = ps[:].rearrange("p (k h c) -> p k h c", k=grp, h=HPC)[
                    :, :, :, 0:E
                ]
                rb = r[:].rearrange("p (k h c) -> p k h c", k=grp, h=HPC)
                numer_b, rb = bass.broadcast_tensor_aps(numer, rb)
                osl = slice(j * HPC * E, (j + grp) * HPC * E)
                oview = ob[:, osl].rearrange("p (k h c) -> p k h c", k=grp, h=HPC)
                nc.vector.scalar_tensor_tensor(
                    oview, numer_b, 1.0, rb, ALU.mult, ALU.mult
                )
                # stream output on the Pool DGE queue: each trigger costs
                # ~600ns of Pool-engine SWDGE time, so keep pieces coarse
                # (one per 8-chunk span) except on the last batch, where
                # smaller per-group pieces shorten the drain.
                if b == B - 1:
                    nc.gpsimd.dma_start(o[b][:, osl], ob[:, osl])
                elif gi % 3 == 2:
                    qsl = slice((j + grp - 8) * HPC * E, (j + grp) * HPC * E)
                    nc.gpsimd.dma_start(o[b][:, qsl], ob[:, qsl])
                j += grp

        emit_load_phi(0)
        emit_mm1(0)
        emit_load_phi(1)
        emit_tail(0)
        emit_mm1(1)
        emit_load_phi(2)
        emit_tail(1)
        emit_mm1(2)
        emit_load_phi(3)
        emit_tail(2)
        emit_mm1(3)
        emit_tail(3)
    nc.finalize()
    return nc


_NC_CACHE = None


def _get_nc():
    global _NC_CACHE
    if _NC_CACHE is None:
        _NC_CACHE = build_nc()
    return _NC_CACHE


def make_in_maps(query, key, value):
    query = np.asarray(query, dtype=np.float32)
    key = np.asarray(key, dtype=np.float32)
    value = np.asarray(value, dtype=np.float32)
    in_maps = []
    for c in range(NCORES):
        lo = c * HPC * E
        hi = lo + HPC * E
        qt = np.ascontiguousarray(
            query[:, :, lo:hi].transpose(0, 2, 1), dtype=np.float16
        )
        kva = np.empty((B, T, W2), np.float16)
        kva[..., 0:KC] = key[:, :, lo:hi]
        kva[..., KC : KC + E] = value[:, :, lo : lo + E]
        kva[..., KC + E] = 1.0
        kva[..., KC + EA : KC + EA + E] = value[:, :, lo + E : hi]
        kva[..., KC + EA + E] = 1.0
        in_maps.append({"qt": qt, "kva": kva})
    return in_maps


def assemble_out(results):
    out = np.empty((B, T, D), np.float32)
    for c in range(NCORES):
        # o[b, p, ((j*2 + h)*64 + e)] = out[b, t=128j+p, c*128 + h*64 + e]
        oc = np.asarray(results[c]["o"], dtype=np.float32)
        oc = oc.reshape(B, P, NJ, HPC, E).transpose(0, 2, 1, 3, 4)
        out[:, :, c * HPC * E : (c + 1) * HPC * E] = oc.reshape(B, T, HPC * E)
    return out


def run(query, key, value, **spmd_kwargs):
    nc = _get_nc()
    in_maps = make_in_maps(query, key, value)
    res = run_bass_kernel_spmd(nc, in_maps, core_ids=list(range(NCORES)), **spmd_kwargs)
    return assemble_out(res.results), res


def kernel(query, key, value):
    out, _ = run(query, key, value)
    return out


# revision 27
# speedup vs baseline: 1.1743x; 1.0348x over previous
"""Linear attention (non-causal, elu+1 feature map) on 8 Trainium2 cores — v8.

Math per (batch b, head h), phi(x) = elu(x)+1:
    C_aug = phi(K)^T @ [V | 1]        # (64, 65): context + k_sum col
    numer = phi(Q) @ C_aug[:, :64]
    denom = phi(Q) @ C_aug[:, 64]
    out   = numer / denom             # eps=1e-6 negligible vs denom ~1e5

Key choices vs the fp32 baseline (233us):
  * fp16 inputs (host casts): PE matmuls at 1 cycle/row instead of 4, one
    LDWEIGHTS pass instead of two, half the HBM traffic (33MB -> 16.3MB per
    core).
  * Both heads fused per matmul. Host packs [K0|K1|V0|1|V1|1] (258 cols per
    t-row) so mm1's stationary (128 K-cols) and moving (130 V-cols) APs are
    single-stride; psum diag blocks give C0_aug/C1_aug. mm2 streams a
    block-diagonal 128x130 C against contiguous 128-col phiQ chunks.
  * phi split balanced across PE and DVE (both measured near-saturated):
      - K: phi materialized in place via min (DVE 4x) / exp (Act) /
        (x+1)max(t) scalar_tensor_tensor (DVE 1x) -> mm1 is 32 matmuls.
      - Q: phi(q) = exp(min(q,0)) + relu(q), never materialized; mm2
        accumulates the E and R passes in psum (2 matmuls/chunk), keeping
        the 1x-rate stt off the DVE for Q at the cost of PE time.
  * Per-queue DMA bandwidth is only ~24GB/s, so every transfer is split into
    ~130-260KB pieces across many queues (input triggers on SP, output
    triggers on the idle Pool DGE so they can't head-of-line block input
    prefetch). Batch 0 is split finer to shorten pipeline fill.
  * normalize: reciprocal_approx_fast (51 ULP, ~5x faster; denom ~1e5 so
    edge cases are impossible) + one stride-0-broadcast scalar_tensor_tensor
    per 3-chunk psum group, streaming each finished group to HBM.
  * Three-stage software pipeline in EMISSION order (engine sequencers are
    in-order): L(b)=loads+phi, M(b)=mm1, B(b)=C-cast+mm2+normalize+output,
    emitted L0 M0 L1 B0 M1 L2 B1 M2 L3 B2 M3 B3: PE order stays
    mm1(b),mm2(b),mm1(b+1); DVE does phi(b+1) before norm(b); Act does
    exp(b+1) before the C-cast of b.

Accuracy: fp16 quantization of phi(K),V gives C entries ~0.2% rms error;
through the normalizer this lands ~1.4e-4 absolute worst-case on outputs vs
the 2e-2 per-element gate with its 1e-3 floor (measured 1.52e-2 max rel).
"""

from contextlib import ExitStack

import numpy as np

import concourse.bacc as bacc
import concourse.bass as bass
import concourse.mybir as mybir
import concourse.tile as tile
from concourse.bass_utils import run_bass_kernel_spmd

B = 4
T = 4096
D = 1024
H = 16
E = 64
EA = E + 1
NCORES = 8
HPC = H // NCORES  # 2 heads per core
KC = HPC * E  # 128 packed K columns per t-row
W2 = KC + HPC * EA  # 258 cols per kva row: [K0|K1|V0|1|V1|1]
P = 128
NT = T // P  # 32 t-tiles for mm1 (t = p*32 + n)
NJ = T // P  # 32 t-chunks for mm2 (t = 128*j + p)
F16 = mybir.dt.float16
F32 = mybir.dt.float32
BF16 = mybir.dt.bfloat16
AF = mybir.ActivationFunctionType
ALU = mybir.AluOpType

# mm2 psum grouping: chunks per tile (3*130*4B = 1560B <= 2KB bank)
GRPS = [3, 3, 2, 3, 3, 2, 3, 3, 2, 3, 3, 2]
assert sum(GRPS) == NJ


def build_nc():
    nc = bacc.Bacc("TRN2", target_bir_lowering=False, debug=False)
    qt = nc.dram_tensor("qt", [B, P, T], F16, kind="ExternalInput").ap()
    kva = nc.dram_tensor("kva", [B, T, W2], F16, kind="ExternalInput").ap()
    o = nc.dram_tensor("o", [B, P, NJ * HPC * E], BF16, kind="ExternalOutput").ap()

    with tile.TileContext(nc) as tc, ExitStack() as ctx:
        qt_pool = ctx.enter_context(tc.tile_pool(name="qt", bufs=3))
        kv_pool = ctx.enter_context(tc.tile_pool(name="kv", bufs=2))
        eq_pool = ctx.enter_context(tc.tile_pool(name="eq", bufs=12))
        tk_pool = ctx.enter_context(tc.tile_pool(name="tk", bufs=8))
        c_pool = ctx.enter_context(tc.tile_pool(name="c", bufs=2))
        out_pool = ctx.enter_context(tc.tile_pool(name="out", bufs=2))
        r_pool = ctx.enter_context(tc.tile_pool(name="r", bufs=8))
        psc_pool = ctx.enter_context(tc.tile_pool(name="psc", bufs=2, space="PSUM"))
        pso_pool = ctx.enter_context(tc.tile_pool(name="pso", bufs=6, space="PSUM"))

        HW = NT * W2  # 8256 elems per partition
        TQ = T // 4  # 1024 cols per phi quarter
        NQ = NT // 4  # 8 n-tiles per phi quarter

        state = {}

        def emit_load_phi(b):
            # Q^T load; E_q = exp(min(q,0)) into eq tiles, R_q = relu(q).
            # DMA pieces sized ~130KB (b=0: finer) so single-queue time stays
            # low; each phi quarter depends only on its own pieces.
            # K side first: mm1 is the earliest PE consumer, so its DMAs and
            # phi must lead the in-order SP/DVE/Act queues.
            # [K0|K1|V0|1|V1|1] load. Even quarters: phi(K) materialized in
            # place via stt (mm1 single pass). Odd quarters: E_k in tk, R_k
            # in place (mm1 double pass) — balances DVE vs PE load.
            kv = kv_pool.tile([P, HW], F16)
            kvr = kv[:].rearrange("p (n c) -> p n c", c=W2)
            tks = {}
            for q4 in range(4):
                for z in range(2):  # 2 pieces/quarter: single-queue ~5.5us
                    w = HW // 8
                    csl = slice(q4 * (HW // 4) + z * w, q4 * (HW // 4) + (z + 1) * w)
                    nc.sync.dma_start(
                        kv[:, csl],
                        kva[b].rearrange("(p n) c -> p (n c)", p=P)[:, csl],
                    )
                nsl = slice(q4 * NQ, (q4 + 1) * NQ)
                kview = kvr[:, nsl, 0:KC]
                tk = tk_pool.tile([P, NQ * KC], F16)
                tk3 = tk[:].rearrange("p (n c) -> p n c", c=KC)
                nc.vector.tensor_scalar_min(tk3, kview, 0.0)
                nc.scalar.activation(tk3, tk3, AF.Exp)
                if q4 % 2 == 0:
                    nc.vector.scalar_tensor_tensor(
                        kview, kview, 1.0, tk3, ALU.add, ALU.max
                    )
                else:
                    nc.vector.tensor_scalar_max(kview, kview, 0.0)
                    tks[q4] = tk

            qt_t = qt_pool.tile([P, T], F16)
            eqs = []
            for q4 in range(4):
                for z in range(2):
                    w = TQ // 2
                    sl = slice(q4 * TQ + z * w, q4 * TQ + (z + 1) * w)
                    nc.sync.dma_start(qt_t[:, sl], qt[b, :, sl])
                sl = slice(q4 * TQ, (q4 + 1) * TQ)
                x = qt_t[:, sl]
                tq = eq_pool.tile([P, TQ], F16)
                nc.vector.tensor_scalar_min(tq[:], x, 0.0)
                nc.scalar.activation(tq[:], tq[:], AF.Exp)
                nc.vector.tensor_scalar_max(x, x, 0.0)
                eqs.append(tq)
            state[b] = (qt_t, eqs, kv, kvr, tks)

        def emit_mm1(b):
            qt_t, eqs, kv, kvr, tks = state[b]
            psc = psc_pool.tile([P, HPC * EA], F32)
            for n in range(NT):
                q4, nq = n // NQ, n % NQ
                if q4 % 2 == 1:
                    nc.tensor.matmul(
                        psc[:],
                        lhsT=tks[q4][:, nq * KC : (nq + 1) * KC],
                        rhs=kvr[:, n, KC:W2],
                        start=(n == 0),
                        stop=False,
                    )
                nc.tensor.matmul(
                    psc[:],
                    lhsT=kvr[:, n, 0:KC],
                    rhs=kvr[:, n, KC:W2],
                    start=(n == 0),  # n=0 is in an even (single-pass) quarter
                    stop=(n == NT - 1),
                )
            # Cast the C diag blocks right here: Act reaches these just as
            # mm1 drains, instead of after the NEXT batch's exp passes, which
            # kept mm2 waiting ~5us per batch on c_sb.
            c_sb = c_pool.tile([P, HPC * EA], F16)
            nc.vector.memset(c_sb[:], 0.0)
            nc.scalar.copy(c_sb[0:E, 0:EA], psc[0:E, 0:EA])
            nc.scalar.copy(c_sb[E:P, EA : 2 * EA], psc[E:P, EA : 2 * EA])
            state[b] = (qt_t, eqs, c_sb)

        def emit_tail(b):
            qt_t, eqs, c_sb = state[b]
            # mm2 (E and R accumulated) + normalize + streamed output
            ob = out_pool.tile([P, NJ * HPC * E], BF16)
            j = 0
            for gi, grp in enumerate(GRPS):
                ps = pso_pool.tile([P, grp * HPC * EA], F32)
                for k in range(grp):
                    jj = j + k
                    q4, jq = jj // 8, jj % 8
                    blk = ps[:, k * HPC * EA : (k + 1) * HPC * EA]
                    nc.tensor.matmul(
                        blk,
                        lhsT=eqs[q4][:, jq * P : (jq + 1) * P],
                        rhs=c_sb[:],
                        start=True,
                        stop=False,
                    )
                    nc.tensor.matmul(
                        blk,
                        lhsT=qt_t[:, jj * P : (jj + 1) * P],
                        rhs=c_sb[:],
                        start=False,
                        stop=True,
                    )
                r = r_pool.tile([P, grp * HPC], F32)
                nc.vector.reciprocal_approx_fast(r[:], ps[:, E::EA])
                numer = ps[:].rearrange("p (k h c) -> p k h c", k=grp, h=HPC)[
                    :, :, :, 0:E
                ]
                rb = r[:].rearrange("p (k h c) -> p k h c", k=grp, h=HPC)
                numer_b, rb = bass.broadcast_tensor_aps(numer, rb)
                osl = slice(j * HPC * E, (j + grp) * HPC * E)
                oview = ob[:, osl].rearrange("p (k h c) -> p k h c", k=grp, h=HPC)
                nc.vector.scalar_tensor_tensor(
                    oview, numer_b, 1.0, rb, ALU.mult, ALU.mult
                )
                # stream output on the Pool DGE queue: each trigger costs
                # ~600ns of Pool-engine SWDGE time, so keep pieces coarse
                # (one per 8-chunk span) except on the last batch, where
                # smaller per-group pieces shorten the drain.
                if b == B - 1:
                    nc.gpsimd.dma_start(o[b][:, osl], ob[:, osl])
                elif gi % 3 == 2:
                    qsl = slice((j + grp - 8) * HPC * E, (j + grp) * HPC * E)
                    nc.gpsimd.dma_start(o[b][:, qsl], ob[:, qsl])
                j += grp

        emit_load_phi(0)
        emit_mm1(0)
        emit_load_phi(1)
        emit_tail(0)
        emit_mm1(1)
        emit_load_phi(2)
        emit_tail(1)
        emit_mm1(2)
        emit_load_phi(3)
        emit_tail(2)
        emit_mm1(3)
        emit_tail(3)
    nc.finalize()
    return nc


_NC_CACHE = None


def _get_nc():
    global _NC_CACHE
    if _NC_CACHE is None:
        _NC_CACHE = build_nc()
    return _NC_CACHE


def make_in_maps(query, key, value):
    query = np.asarray(query, dtype=np.float32)
    key = np.asarray(key, dtype=np.float32)
    value = np.asarray(value, dtype=np.float32)
    in_maps = []
    for c in range(NCORES):
        lo = c * HPC * E
        hi = lo + HPC * E
        qt = np.ascontiguousarray(
            query[:, :, lo:hi].transpose(0, 2, 1), dtype=np.float16
        )
        kva = np.empty((B, T, W2), np.float16)
        kva[..., 0:KC] = key[:, :, lo:hi]
        kva[..., KC : KC + E] = value[:, :, lo : lo + E]
        kva[..., KC + E] = 1.0
        kva[..., KC + EA : KC + EA + E] = value[:, :, lo + E : hi]
        kva[..., KC + EA + E] = 1.0
        in_maps.append({"qt": qt, "kva": kva})
    return in_maps


def assemble_out(results):
    out = np.empty((B, T, D), np.float32)
    for c in range(NCORES):
        # o[b, p, ((j*2 + h)*64 + e)] = out[b, t=128j+p, c*128 + h*64 + e]
        oc = np.asarray(results[c]["o"], dtype=np.float32)
        oc = oc.reshape(B, P, NJ, HPC, E).transpose(0, 2, 1, 3, 4)
        out[:, :, c * HPC * E : (c + 1) * HPC * E] = oc.reshape(B, T, HPC * E)
    return out


def run(query, key, value, **spmd_kwargs):
    nc = _get_nc()
    in_maps = make_in_maps(query, key, value)
    res = run_bass_kernel_spmd(nc, in_maps, core_ids=list(range(NCORES)), **spmd_kwargs)
    return assemble_out(res.results), res


def kernel(query, key, value):
    out, _ = run(query, key, value)
    return out
